# revision 19
# baseline (speedup 1.0000x reference)
"""CharRNN Trainium2 kernel.

Data-parallel over batch across 8 NeuronCores (16 batch rows per core,
small weights replicated). The sequential recurrence keeps the hidden
state transposed ("hT": hidden dim on partitions, packed columns
(step, jchunk, batch)) so the per-step matmuls consume and produce the
same layout with no per-step transposes.

Math per core (B=16 batch rows, H=512, V=E=128, L=1024):
  E2'[v, h]   = (embedding @ W_ih.T)[v, h] + b_h[h]          (setup)
  onehotT     = (x[c] == v)                                  (iota compare)
  xinT[h, c]  = (E2'.T @ onehotT)[h, c]                      (phase 1, SBUF-resident)
  hT(t+1)     = tanh(sum_k W_hh[j,k] @ hT_k(t) + xinT_t)     (phase 2, serial)
  logits      = hT.T @ W_ho.T + b_o                          (phase 3, interleaved)

Per recurrence step: an identity-matmul opens each PSUM accumulation
group (start=True clears the bank and deposits xin), 16 W_hh matmuls
accumulate, and two [128, 32] tanh ops drain PSUM -> hT.  hT lives in a
2-deep ring of 128-step SBUF blocks; logits tiles for a finished block
are interleaved into the PE slack of later steps.  Everything bf16 on
the PE except PSUM accumulation (fp32) and final logits (fp32).
"""

import sys

sys.path.insert(0, "/opt/trn_rl_repo")

import numpy as np

from concourse import bass, tile, mybir
from concourse.bass_utils import run_bass_kernel_spmd

F32 = mybir.dt.float32
BF16 = mybir.dt.bfloat16
I16 = mybir.dt.int16

VOCAB = 128
EMBED = 128
HIDDEN = 512
BATCH = 128
SEQLEN = 1024
NCORES = 8
BPC = BATCH // NCORES  # 16 batch rows per core
NJ = HIDDEN // 128  # 4 hidden chunks
BLK = 128  # recurrence steps per hT SBUF block

Tanh = mybir.ActivationFunctionType.Tanh
Alu = mybir.AluOpType


def split_multi_waits(nc):
    """This container's walrus supports one sync-wait per instruction; hoist
    extra waits into standalone EventSemaphore instructions just before."""
    n_split = 0
    for f in nc.m.functions:
        for b in f.blocks:
            new_instrs = []
            for ins in b.instructions:
                si = ins.sync_info
                waits = list(si.on_wait) if (si is not None and si.on_wait) else []
                if len(waits) > 1:
                    n_split += 1
                    for idx, w in enumerate(waits[:-1]):
                        ev = mybir.InstEventSemaphore(
                            name=f"{ins.name}-wsplit{idx}", ins=[], outs=[]
                        )
                        ev.engine = ins.engine
                        ev.sync_info = mybir.SyncInfo(on_wait=[w], on_update=[])
                        new_instrs.append(ev)
                    ins.sync_info = mybir.SyncInfo(
                        on_wait=[waits[-1]], on_update=list(si.on_update)
                    )
                new_instrs.append(ins)
            b.instructions = new_instrs
    return n_split


def build_nc(L=SEQLEN, repeat=1):
    C = BPC * L  # total (t, b) columns per core
    G = (L + BLK - 1) // BLK  # number of hT blocks
    nc = bass.Bass(trn_type="TRN2")

    # ---- I/O ----
    xb_d = nc.declare_dram_parameter("xb", [128, C], I16, isOutput=False)
    hid_d = nc.declare_dram_parameter("hidden", [BPC, HIDDEN], F32, isOutput=False)
    emb_d = nc.declare_dram_parameter("embedding", [VOCAB, EMBED], F32, isOutput=False)
    wih_d = nc.declare_dram_parameter("W_ih", [HIDDEN, EMBED], F32, isOutput=False)
    whh_d = nc.declare_dram_parameter("W_hh", [HIDDEN, HIDDEN], F32, isOutput=False)
    bh_d = nc.declare_dram_parameter("b_h", [1, HIDDEN], F32, isOutput=False)
    who_d = nc.declare_dram_parameter("W_ho", [VOCAB, HIDDEN], F32, isOutput=False)
    bo_d = nc.declare_dram_parameter("b_o", [1, VOCAB], F32, isOutput=False)
    logits_d = nc.declare_dram_parameter("logits", [BPC, L, VOCAB], F32, isOutput=True)
    hout_d = nc.declare_dram_parameter("hidden_out", [BPC, HIDDEN], F32, isOutput=True)

    with tile.TileContext(nc) as tc:
        with tc.tile_pool(name="const", bufs=1) as const:
            ones_s = const.tile([128, 128], F32, tag="ones")
            ident_s = const.tile([128, 128], F32, tag="ident")
            identb_s = const.tile([128, 128], BF16, tag="identb")
            iota_s = const.tile([128, 1], F32, tag="iota")
            wt_s = const.tile([128, 16 * 128], BF16, tag="wt")  # W_hh.T tiles (j,k)
            whoT_s = const.tile([128, HIDDEN], BF16, tag="whoT")  # W_ho.T
            e2_s = const.tile([128, HIDDEN], BF16, tag="e2")  # E2' bf16
            bh_s = const.tile([1, HIDDEN], F32, tag="bh")
            bo_s = const.tile([1, VOCAB], F32, tag="bo")
            xin_all = const.tile([128, 64 * L], BF16, tag="xin")  # (t, j, b) cols
            ht0 = const.tile([128, 64], BF16, tag="ht0")  # slot 0

            nc.vector.memset(ones_s[:], 1.0)
            # identity = (p - c == 0) ? 1 : 0
            nc.gpsimd.affine_select(
                out=ident_s[:],
                in_=ones_s[:],
                pattern=[[-1, 128]],
                compare_op=Alu.is_equal,
                fill=0.0,
                base=0,
                channel_multiplier=1,
            )
            nc.vector.tensor_copy(identb_s[:], ident_s[:])
            nc.gpsimd.iota(
                iota_s[:],
                pattern=[[0, 1]],
                base=0,
                channel_multiplier=1,
                allow_small_or_imprecise_dtypes=True,
            )
            nc.sync.dma_start(bh_s[:], bh_d[:])
            nc.sync.dma_start(bo_s[:], bo_d[:])

            # ---- setup: load + transpose weights ----
            with (
                tc.tile_pool(name="setup", bufs=2) as setup,
                tc.tile_pool(name="pset", bufs=2, space="PSUM") as pset,
            ):
                # W_hh.T tiles: wt[:, (j*4+k)*128+kk, mm] = W_hh[j*128+mm, k*128+kk]
                for j in range(NJ):
                    whh_j = setup.tile([128, HIDDEN], F32, tag="whhj")
                    nc.sync.dma_start(whh_j[:], whh_d[j * 128 : (j + 1) * 128, :])
                    for k in range(NJ):
                        pt = pset.tile([128, 128], F32, tag="pt")
                        nc.tensor.transpose(
                            pt[:], whh_j[:, k * 128 : (k + 1) * 128], ident_s[:]
                        )
                        idx = j * 4 + k
                        nc.vector.tensor_copy(
                            wt_s[:, idx * 128 : (idx + 1) * 128], pt[:]
                        )
                # W_ho.T chunks: whoT_s[hh, k*128+v] = W_ho[v, k*128+hh]
                who_s = setup.tile([128, HIDDEN], F32, tag="whos")
                nc.sync.dma_start(who_s[:], who_d[:])
                for k in range(NJ):
                    pt = pset.tile([128, 128], F32, tag="pt")
                    nc.tensor.transpose(
                        pt[:], who_s[:, k * 128 : (k + 1) * 128], ident_s[:]
                    )
                    nc.vector.tensor_copy(whoT_s[:, k * 128 : (k + 1) * 128], pt[:])
                # embT, W_ihT -> E2' = emb @ W_ih.T + b_h   [v, h]
                emb_s = setup.tile([128, 128], F32, tag="embs")
                embT_s = setup.tile([128, 128], F32, tag="embT")
                wihT_s = setup.tile([128, HIDDEN], F32, tag="wihT")
                nc.sync.dma_start(emb_s[:], emb_d[:])
                pt = pset.tile([128, 128], F32, tag="pt")
                nc.tensor.transpose(pt[:], emb_s[:], ident_s[:])
                nc.vector.tensor_copy(embT_s[:], pt[:])
                for j in range(NJ):
                    wih_j = setup.tile([128, EMBED], F32, tag="wihj")
                    nc.sync.dma_start(wih_j[:], wih_d[j * 128 : (j + 1) * 128, :])
                    pt = pset.tile([128, 128], F32, tag="pt")
                    nc.tensor.transpose(pt[:], wih_j[:], ident_s[:])
                    nc.vector.tensor_copy(wihT_s[:, j * 128 : (j + 1) * 128], pt[:])
                pe2 = pset.tile([128, HIDDEN], F32, tag="pe2")
                nc.tensor.matmul(pe2[:], embT_s[:], wihT_s[:], start=True, stop=False)
                nc.tensor.matmul(
                    pe2[:], ones_s[0:1, :], bh_s[:], start=False, stop=True
                )
                nc.vector.tensor_copy(e2_s[:], pe2[:])

                # init slot 0 from the (transposed) initial hidden state
                hid_s = setup.tile([BPC, HIDDEN], F32, tag="hids")
                nc.sync.dma_start(hid_s[:], hid_d[:])
                for k in range(NJ):
                    pt2 = pset.tile([128, BPC], F32, tag="pt2")
                    nc.tensor.transpose(
                        pt2[:], hid_s[:, k * 128 : (k + 1) * 128], ident_s[0:BPC, 0:BPC]
                    )
                    nc.vector.tensor_copy(ht0[:, k * BPC : (k + 1) * BPC], pt2[:])

            # ---- phase 1: xin_all = E2'.T @ onehot(x), straight into SBUF --
            # emitted before phase 2; overlaps it at runtime (subtile deps)
            CHUNK = min(1024, C)  # (t, b) columns per chunk = 64 steps
            xin_v = xin_all.rearrange("p (t j b) -> p t j b", j=NJ, b=BPC)
            with (
                tc.tile_pool(name="p1", bufs=2) as p1,
                tc.tile_pool(name="pp1", bufs=2, space="PSUM") as pp1,
            ):
                for ci, c0 in enumerate(range(0, C, CHUNK)):
                    xb_c = p1.tile([128, CHUNK], I16, tag="xbc")
                    oh_c = p1.tile([128, CHUNK], BF16, tag="ohc")
                    nc.sync.dma_start(xb_c[:], xb_d[:, c0 : c0 + CHUNK])
                    nc.vector.tensor_scalar(
                        oh_c[:], xb_c[:], iota_s[:, 0:1], None, Alu.is_equal
                    )
                    for j in range(NJ):
                        for s0 in range(0, CHUNK, 512):
                            px = pp1.tile([128, 512], F32, tag="px")
                            nc.tensor.matmul(
                                px[:],
                                e2_s[:, j * 128 : (j + 1) * 128],
                                oh_c[:, s0 : s0 + 512],
                                start=True,
                                stop=True,
                            )
                            t0 = (c0 + s0) // BPC  # 32 steps per psum
                            dst = xin_v[:, t0 : t0 + 512 // BPC, j, :]
                            src = px.rearrange("p (t b) -> p t b", b=BPC)
                            nc.vector.tensor_copy(dst, src)

            # ---- phases 2+3 interleaved ----
            blk = []

            def slot(s, k=None):
                if s == 0:
                    t_, col = ht0, 0
                else:
                    t_, col = blk[(s - 1) // BLK], ((s - 1) % BLK) * 64
                if k is None:
                    return t_[:, col : col + 64]
                return t_[:, col + k * 16 : col + (k + 1) * 16]

            with (
                tc.tile_pool(name="p23", bufs=3) as p23,
                tc.tile_pool(name="hpool", bufs=2) as hpool,
                tc.tile_pool(name="pp2", bufs=4, space="PSUM") as pp2,
                tc.tile_pool(name="pp3", bufs=2, space="PSUM") as pp3,
            ):

                def logits_tile(g, b):
                    """logits[b, g*BLK : g*BLK+Mt, :] from hT block g."""
                    Mt = min(BLK, L - g * BLK)
                    hv = blk[g].rearrange("p (t j b) -> p t j b", j=NJ, b=BPC)
                    pl = pp3.tile([BLK, VOCAB], F32, tag="pl", name="pl")
                    for k in range(NJ):
                        nc.tensor.matmul(
                            pl[0:Mt, :],
                            hv[:, 0:Mt, k, b],
                            whoT_s[:, k * 128 : (k + 1) * 128],
                            start=(k == 0),
                            stop=False,
                        )
                    nc.tensor.matmul(
                        pl[0:Mt, :],
                        ones_s[0:1, 0:Mt],
                        bo_s[:],
                        start=False,
                        stop=True,
                    )
                    ls = p23.tile([BLK, VOCAB], F32, tag="ls", name="ls")
                    nc.vector.tensor_copy(ls[0:Mt, :], pl[0:Mt, :])
                    nc.sync.dma_start(
                        logits_d[b, g * BLK : g * BLK + Mt, :], ls[0:Mt, :]
                    )

                # spread block g's 16 logits tiles over steps right after the
                # block finishes: tile (g, b) after step (g+1)*BLK + 4*b
                sched = {}
                for g in range(G):
                    for b in range(BPC):
                        s_e = (g + 1) * BLK + 4 * b
                        if s_e < L:
                            sched.setdefault(s_e, []).append((g, b))

                def emit_phase23():
                    blk.clear()
                    for t in range(L):
                        if t % BLK == 0:
                            blk.append(
                                hpool.tile(
                                    [128, 64 * BLK], BF16, tag="hblk", name="hblk"
                                )
                            )
                        # recurrence step t -> slot t+1
                        for g in range(2):
                            ph = pp2.tile([128, 2, BPC], F32, tag="ph", name="ph")
                            phv = ph.rearrange("p a b -> p (a b)")
                            nc.tensor.matmul(
                                phv,
                                identb_s[:],
                                xin_all[:, t * 64 + g * 32 : t * 64 + (g + 1) * 32],
                                start=True,
                                stop=False,
                                skip_group_check=True,
                            )
                            for k in range(NJ):
                                for jj in range(2):
                                    idx = (2 * g + jj) * 4 + k
                                    nc.tensor.matmul(
                                        ph[:, jj, :],
                                        wt_s[:, idx * 128 : (idx + 1) * 128],
                                        slot(t, k),
                                        start=False,
                                        stop=(k == 3 and jj == 1),
                                        skip_group_check=True,
                                    )
                            nc.scalar.activation(
                                slot(t + 1)[:, g * 32 : (g + 1) * 32], phv, Tanh
                            )
                        # interleave finished blocks' logits tiles into PE slack
                        for g, b in sched.get(t, []):
                            logits_tile(g, b)

                    # remaining logits tiles (last block; and short-L leftovers)
                    done = {gb for v in sched.values() for gb in v}
                    for g in range(G):
                        for b in range(BPC):
                            if (g, b) not in done:
                                logits_tile(g, b)

                if repeat > 1:
                    with tc.For_i(0, repeat, 1):
                        emit_phase23()
                else:
                    emit_phase23()

                # final hidden back to [b, h] fp32
                hstage = p23.tile([BPC, HIDDEN], F32, tag="hstage")
                for k in range(NJ):
                    ptf = pp3.tile([BPC, 128], BF16, tag="ptf", name="ptf")
                    nc.tensor.transpose(ptf[:], slot(L, k), identb_s[:])
                    nc.vector.tensor_copy(hstage[:, k * 128 : (k + 1) * 128], ptf[:])
                nc.sync.dma_start(hout_d[:], hstage[:])

    split_multi_waits(nc)
    return nc


_cache = {}


def _get_nc(L):
    if L not in _cache:
        _cache[L] = build_nc(L)
    return _cache[L]


def kernel(x, hidden, embedding, W_ih, W_hh, b_h, W_ho, b_o, _L=None):
    x = np.asarray(x)
    hidden = np.asarray(hidden, dtype=np.float32)
    L = int(x.shape[1]) if _L is None else _L
    nc = _get_nc(L)

    weights = {
        "embedding": np.ascontiguousarray(embedding, dtype=np.float32),
        "W_ih": np.ascontiguousarray(W_ih, dtype=np.float32),
        "W_hh": np.ascontiguousarray(W_hh, dtype=np.float32),
        "b_h": np.ascontiguousarray(np.reshape(b_h, (1, HIDDEN)).astype(np.float32)),
        "W_ho": np.ascontiguousarray(W_ho, dtype=np.float32),
        "b_o": np.ascontiguousarray(np.reshape(b_o, (1, VOCAB)).astype(np.float32)),
    }
    in_maps = []
    for c in range(NCORES):
        xs = x[c * BPC : (c + 1) * BPC, :L].astype(np.int16)  # [16, L]
        xf = np.ascontiguousarray(xs.T).reshape(1, -1)  # t-major cols
        xb = np.ascontiguousarray(np.broadcast_to(xf, (128, BPC * L)))
        in_maps.append(
            {
                "xb": xb,
                "hidden": np.ascontiguousarray(hidden[c * BPC : (c + 1) * BPC]),
                **weights,
            }
        )

    res = run_bass_kernel_spmd(nc, in_maps, core_ids=list(range(NCORES)))
    logits = np.concatenate([r["logits"] for r in res.results], axis=0)
    final_hidden = np.concatenate([r["hidden_out"] for r in res.results], axis=0)
    return logits, final_hidden
